# revision 43
# baseline (speedup 1.0000x reference)
"""Trainium2 Bass kernel for GQA attention (B=2, S=2048, D=2048, H=32, KV=8, HD=64).

Sharding over 8 NeuronCores: batch (2) x 4-way head tensor-parallel.
Core c handles batch c//4 and KV heads {2r, 2r+1} (r = c%4) with their
8 query heads. After attention, 4-core AllGathers (one per head-pair
half) assemble the full attention output (transposed layout) and each
core computes a 512-column shard of the final wo projection.

All matmuls run in bf16 (inputs converted host-side), accumulation fp32.

Layout tricks (host-side permutations, cancel out in the math):
- wq/wk columns are permuted inside each head's 64-dim block so rope pairs
  (even, odd) become (first-32, last-32) contiguous partition blocks.
- wq columns are ordered so QT tile t holds query head (g0, rep t) in
  partitions 0-63 and (g1, rep t) in partitions 64-127, which lets the
  scores matmuls for the two heads pack into disjoint PE row groups.
- wo rows are permuted to match the AllGather'ed attention-transposed
  row order.

Pipeline structure:
- scores for 2 chunks x 2 heads land in one 4-bank PSUM tile, so each
  exp ACTIVATE covers 2048 elems/partition (amortizes the ~352-cycle
  per-op overhead).
- a depth-2 scores/exp/PV software pipeline keeps the PE from stalling
  on the single-buffered scores PSUM, and QT[t+1] projection matmuls
  are interleaved into attention pair t as PE filler work (the K
  projection instead runs 4 accumulators wide so the PE tracks the xT
  DMA stream during startup).
- softmax denominators come free via a ones-column appended to V;
  normalization (copy/fast-recip/broadcast/mult) runs off the critical
  path after the PV accumulators are drained to SBUF.
- diagonal (causal-boundary) super-chunks compute only the live q
  window (half width), with a triangular mask multiply on the rest.
- the wo projection splits into seq-row halves: the lo half waits only
  on the pairs' first AllGathers and overlaps the hi-half gathers.
"""

import numpy as np
import ml_dtypes

import concourse.bass as bass
import concourse.mybir as mybir
import concourse.tile as tile
from concourse import bacc
from concourse.bass_utils import run_bass_kernel_spmd

B, S, D = 2, 2048, 2048
H, KV, HD = 32, 8, 64
NREP = H // KV
P = 128
NCORES = 8
GRP = 4                  # cores per batch group
QCOLS = 8 * HD           # 512 query cols per core
KCOLS = 2 * HD           # 128 k/v cols per core
OCOLS = D // GRP         # 512 output cols per core
DCH = D // P             # 16 contraction chunks
NJQ = S // 512           # 4 q windows
NPAIR = 4                # head pairs per core (one per QT tile)

bf16 = mybir.dt.bfloat16
f32 = mybir.dt.float32
MULT = mybir.AluOpType.mult
ADD = mybir.AluOpType.add
EXP = mybir.ActivationFunctionType.Exp

_BF = ml_dtypes.bfloat16


def _rope(nc, rp, dst, ps, cosw, sinw):
    """dst = ps * cos + swap32(ps) * sin  (rope in pair-split layout)."""
    n = ps.shape[-1]
    ra = rp.tile([P, n], f32, tag="ra", name="ra", bufs=1)
    rb = rp.tile([P, n], f32, tag="rb", name="rb", bufs=1)
    nc.vector.tensor_tensor(out=ra[:], in0=ps[:], in1=cosw, op=MULT)
    for ob, ib in ((0, 32), (32, 0), (64, 96), (96, 64)):
        nc.vector.tensor_tensor(
            out=rb[ob : ob + 32, :],
            in0=ps[ib : ib + 32, :],
            in1=sinw[ob : ob + 32, :],
            op=MULT,
        )
    nc.vector.tensor_tensor(out=dst, in0=ra[:], in1=rb[:], op=ADD)


def build_graph():
    nc = bacc.Bacc("TRN2", target_bir_lowering=False, debug=False, num_devices=NCORES)

    xT = nc.dram_tensor("xT", [D, S], bf16, kind="ExternalInput")
    wq = nc.dram_tensor("wq", [D, QCOLS], bf16, kind="ExternalInput")
    wk = nc.dram_tensor("wk", [D, KCOLS], bf16, kind="ExternalInput")
    wv = nc.dram_tensor("wv", [D, KCOLS], bf16, kind="ExternalInput")
    wo = nc.dram_tensor("wo", [H * HD, OCOLS], bf16, kind="ExternalInput")
    cos4 = nc.dram_tensor("cos4", [P, S], bf16, kind="ExternalInput")
    sin4 = nc.dram_tensor("sin4", [P, S], bf16, kind="ExternalInput")
    cmask = nc.dram_tensor("cmask", [P, P], bf16, kind="ExternalInput")
    out = nc.dram_tensor("out", [S, OCOLS], f32, kind="ExternalOutput")

    with tile.TileContext(nc) as tc:
        _build_body(tc, nc, xT, wq, wk, wv, wo, cos4, sin4, cmask, out)
    nc.compile()
    return nc


def _build_body(tc, nc, xT, wq, wk, wv, wo, cos4, sin4, cmask, out):
    from contextlib import ExitStack

    with ExitStack() as ctx:
        const = ctx.enter_context(tc.tile_pool(name="const", bufs=1))
        dram = ctx.enter_context(tc.tile_pool(name="dram", bufs=1, space="DRAM"))

        # weights on the gpsimd DMA queue so the sync queue starts on xT
        # immediately (DMA issue is ~0.6us each and serializes per queue)
        wk_sb = const.tile([P, DCH, KCOLS], bf16)
        wv_sb = const.tile([P, DCH, KCOLS], bf16)
        for c in range(DCH):
            nc.gpsimd.dma_start(wk_sb[:, c, :], wk.ap()[c * P : (c + 1) * P, :])
        for c in range(DCH):
            nc.gpsimd.dma_start(wv_sb[:, c, :], wv.ap()[c * P : (c + 1) * P, :])
        mask_sb = const.tile([P, 1, P], bf16)
        nc.gpsimd.dma_start(mask_sb[:, 0, :], cmask.ap())

        # long-lived activation tensors
        QT = [const.tile([P, S], bf16, name=f"qt{t}") for t in range(NPAIR)]
        KT = const.tile([P, S], bf16, name="kt")
        V = const.tile([P, DCH, 130], bf16, name="vsb")  # [g0 64 | 1 | g1 64 | 1]
        attT = [const.tile([P, S], bf16, name=f"attT{t}") for t in range(NPAIR)]

        nc.vector.memset(V[:, :, 64], 1.0)
        nc.vector.memset(V[:, :, 129], 1.0)

        # body-wide PSUM pool: pj0/pj1 are the projection accumulators (2
        # banks); attention adds sAB (4 banks) + otA/otB (2 banks) = 8.
        pps = ctx.enter_context(tc.tile_pool(name="pps", bufs=1, space="PSUM"))
        scp = ctx.enter_context(tc.tile_pool(name="scps", bufs=1, space="PSUM"))
        ex = ctx.enter_context(tc.tile_pool(name="ex", bufs=1))
        nrm = ctx.enter_context(tc.tile_pool(name="nrm", bufs=1))

        proj_ctx = ExitStack()
        proj = proj_ctx.enter_context(tc.tile_pool(name="proj", bufs=1))
        # xT chunks split in column halves over the sync and scalar
        # sequencers: more transfers in flight earlier, and the K projection
        # can start on a half-landed chunk
        xt = []
        xq = (nc.sync, nc.scalar)
        qi = 0
        for c in range(DCH):
            t_ = proj.tile([P, S], bf16, name=f"x{c}", tag=f"x{c}")
            for h in range(2):
                hw_ = slice(h * 1024, (h + 1) * 1024)
                xq[qi % 2].dma_start(t_[:, hw_], xT.ap()[c * P : (c + 1) * P, hw_])
                qi += 1
            xt.append(t_)
        # cos/sin ship as bf16 (halves their HBM traffic competing with the
        # xT stream) and the otherwise-idle scalar engine upconverts to f32
        cos_bf = proj.tile([P, S], bf16)
        nc.gpsimd.dma_start(cos_bf[:], cos4.ap())
        sin_bf = proj.tile([P, S], bf16)
        nc.gpsimd.dma_start(sin_bf[:], sin4.ap())
        cos_sb = proj.tile([P, S], f32)
        sin_sb = proj.tile([P, S], f32)
        for j in range(NJQ):
            sw = slice(j * 512, (j + 1) * 512)
            nc.scalar.copy(cos_sb[:, sw], cos_bf[:, sw])
            nc.scalar.copy(sin_sb[:, sw], sin_bf[:, sw])
        wq_sb = const.tile([P, DCH, QCOLS], bf16)
        for c in range(DCH):
            nc.gpsimd.dma_start(wq_sb[:, c, :], wq.ap()[c * P : (c + 1) * P, :])

        def emit_qt(ot):
            """Generator emitting QT[ot] projection in small PE batches."""
            for jp in range(2):
                ps = [pps.tile([P, 512], f32, tag=f"pj{j}", name="qps", bufs=1)
                      for j in range(2)]
                jss = (2 * jp, 2 * jp + 1)
                for c in range(DCH):
                    for i, js in enumerate(jss):
                        nc.tensor.matmul(
                            ps[i][:], wq_sb[:, c, ot * P : (ot + 1) * P],
                            xt[c][:, js * 512 : (js + 1) * 512],
                            start=(c == 0), stop=(c == DCH - 1),
                        )
                    if c % 2 == 1:
                        yield None
                for i, js in enumerate(jss):
                    sw = slice(js * 512, (js + 1) * 512)
                    _rope(nc, proj, QT[ot][:, sw], ps[i], cos_sb[:, sw],
                          sin_sb[:, sw])
                    yield None

        # ---- phase A: K, V, QT0 projections ------------------------
        # K uses all four free accumulator banks so its 4 matmuls per xT
        # chunk keep the PE fed at the DMA stream rate
        ktags = ("pj0", "pj1", "otA", "otB")
        kps = [pps.tile([P, 512], f32, tag=ktags[j], name="kps", bufs=1)
               for j in range(4)]
        for c in range(DCH):
            for js in range(4):
                nc.tensor.matmul(
                    kps[js][:], wk_sb[:, c, :],
                    xt[c][:, js * 512 : (js + 1) * 512],
                    start=(c == 0), stop=(c == DCH - 1),
                )
        for js in range(4):
            sw = slice(js * 512, (js + 1) * 512)
            _rope(nc, proj, KT[:, sw], kps[js], cos_sb[:, sw], sin_sb[:, sw])
        # V accumulates in the (still unused) scores PSUM banks, so it never
        # write-after-read collides with the K rope draining kps (pj*/ot*) —
        # V and QT0 matmuls can then interleave freely
        vps = scp.tile([P, 4, P], f32, tag="sAB", name="vps", bufs=1)
        for it in range(DCH):
            vp = vps[:, it % 4, :]
            for c in range(DCH):
                nc.tensor.matmul(
                    vp, xt[c][:, it * P : (it + 1) * P], wv_sb[:, c, :],
                    start=(c == 0), stop=(c == DCH - 1),
                )
            nc.scalar.copy(V[:, it, 0:64], vp[:, 0:64])
            nc.scalar.copy(V[:, it, 65:129], vp[:, 64:128])
        for _ in emit_qt(0):
            pass

        # ---- phase B: attention, QT[t+1] interleaved ----------------
        att_loc = [[dram.tile([P, 1024], bf16, name=f"aloc{t}_{h}") for h in range(2)]
                   for t in range(NPAIR)]
        att_all = [[dram.tile([GRP * P, 1024], bf16, name=f"aall{t}_{h}") for h in range(2)]
                   for t in range(NPAIR)]
        wos = None
        wo_sb = None
        chunks = [[None] * 2 for _ in range(16)]

        def emit_norm(pair, jq, otA, otB):
            qw = slice(jq * 512, (jq + 1) * 512)
            # free the single-buffered PV psums quickly via SBUF copies
            unA = nrm.tile([64, 512], bf16, tag="unA", name="unA", bufs=2)
            nc.vector.tensor_copy(out=unA[:], in_=otA[0:64, :])
            denA = nrm.tile([1, 512], f32, tag="denA", name="denA", bufs=2)
            nc.vector.tensor_copy(out=denA[:], in_=otA[64:65, :])
            unB = nrm.tile([64, 512], bf16, tag="unB", name="unB", bufs=2)
            nc.vector.tensor_copy(out=unB[:], in_=otB[0:64, :])
            denB = nrm.tile([1, 512], f32, tag="denB", name="denB", bufs=2)
            nc.vector.tensor_copy(out=denB[:], in_=otB[64:65, :])
            recA = nrm.tile([1, 512], f32, tag="recA", name="recA", bufs=2)
            nc.vector.reciprocal_approx_fast(out=recA[:], in_=denA[:])
            recB = nrm.tile([1, 512], f32, tag="recB", name="recB", bufs=2)
            nc.vector.reciprocal_approx_fast(out=recB[:], in_=denB[:])
            bcA = nrm.tile([64, 512], f32, tag="bcA", name="bcA", bufs=2)
            nc.gpsimd.partition_broadcast(bcA[:], recA[:])
            bcB = nrm.tile([64, 512], f32, tag="bcB", name="bcB", bufs=2)
            nc.gpsimd.partition_broadcast(bcB[:], recB[:])
            nc.vector.tensor_tensor(
                out=attT[pair][0:64, qw], in0=unA[:], in1=bcA[:], op=MULT,
            )
            nc.vector.tensor_tensor(
                out=attT[pair][64:128, qw], in0=unB[:], in1=bcB[:], op=MULT,
            )
            if jq % 2 == 1:  # half complete -> ship + gather
                h = jq // 2
                hw_ = slice(h * 1024, (h + 1) * 1024)
                nc.sync.dma_start(att_loc[pair][h][:], attT[pair][:, hw_])
                nc.gpsimd.collective_compute(
                    "AllGather",
                    mybir.AluOpType.bypass,
                    replica_groups=[[0, 1, 2, 3], [4, 5, 6, 7]],
                    ins=[att_loc[pair][h][:].opt()],
                    outs=[att_all[pair][h][:].opt()],
                )
                if wos is not None:  # pair 3: preload its wo chunks
                    for rr in range(GRP):
                        nc.gpsimd.dma_start(
                            chunks[4 * pair + rr][h][:],
                            att_all[pair][h][rr * P : (rr + 1) * P, :],
                        )

        def emit_pv(ent):
            eAB2_p, base, otA, otB, nch, pair, jq, W, slots = ent
            for j in (0, 1):
                ik = base + j
                first = ik == 0
                last = ik == nch - 1
                d = ik - 4 * jq
                Wc = W if d < 0 else 128 * d  # per-chunk causal window
                o = Wc - W
                sA, sB = slots[j]
                nc.tensor.matmul(
                    otA[0:65, Wc:512], V[:, ik, 0:65], eAB2_p[:, sA, o:],
                    start=first, stop=last,
                )
                nc.tensor.matmul(
                    otB[0:65, Wc:512], V[:, ik, 65:130], eAB2_p[:, sB, o:],
                    start=first, stop=last,
                )
            if base + 2 >= nch:  # last chunks of this (pair, jq)
                emit_norm(pair, jq, otA, otB)

        def emit_wo_loads():
            # wo weights + gathered-chunk preloads, spread across pair 3's
            # supers so the in-order gpsimd stream (which also carries the
            # partition_broadcasts and AllGather triggers) never backs up
            for c in range(DCH):
                nc.gpsimd.dma_start(
                    wo_sb[:, c, :], wo.ap()[c * P : (c + 1) * P, :]
                )
                if c % 2 == 1:
                    yield None
            for t in range(3):
                for h in range(2):
                    for rr in range(GRP):
                        nc.gpsimd.dma_start(
                            chunks[4 * t + rr][h][:],
                            att_all[t][h][rr * P : (rr + 1) * P, :],
                        )
                    yield None

        pend = []  # global software pipeline: scores/exp run 1 ahead of PV
        filler = None
        for pair in range(NPAIR):
            filler = emit_qt(pair + 1) if pair + 1 < NPAIR else emit_wo_loads()
            for jq in range(NJQ):
                otA = pps.tile([P, 512], f32, tag="otA", name="otA", bufs=1)
                otB = pps.tile([P, 512], f32, tag="otB", name="otB", bufs=1)
                nch = 4 * jq + 4
                for sc in range(nch // 2):
                    d0 = 2 * sc - 4 * jq
                    if d0 == 2:
                        W, NW = 256, 256
                        slots = [(0, 2), (1, 3)]
                    else:
                        W, NW = 0, 512
                        slots = [(0, 1), (2, 3)]
                    qwW = slice(jq * 512 + W, (jq + 1) * 512)
                    sAB2 = scp.tile([P, 4, NW], f32, tag="sAB", name="sAB", bufs=1)
                    for j in (0, 1):
                        ik = 2 * sc + j
                        kt_ = slice(ik * P, (ik + 1) * P)
                        d = ik - 4 * jq
                        Wc = W if d < 0 else 128 * d
                        o = Wc - W
                        qwc = slice(jq * 512 + Wc, (jq + 1) * 512)
                        sA, sB = slots[j]
                        nc.tensor.matmul(
                            sAB2[:, sA, o:], KT[0:64, kt_],
                            QT[pair][0:64, qwc], start=True, stop=True,
                        )
                        nc.tensor.matmul(
                            sAB2[:, sB, o:], KT[64:128, kt_],
                            QT[pair][64:128, qwc], start=True, stop=True,
                        )
                    eAB2 = ex.tile([P, 4, NW], bf16, tag="eAB", name="eAB", bufs=5)
                    nc.scalar.activation(eAB2[:], sAB2[:], EXP, scale=0.125)
                    for j in (0, 1):
                        d = 2 * sc + j - 4 * jq
                        if d >= 0:  # diagonal chunk: mask only the triangle
                            o = 128 * d - W
                            sA, sB = slots[j]
                            if sB == sA + 1:
                                msl = eAB2[:, sA : sA + 2, o : o + 128]
                            else:
                                msl = eAB2[:, sA : sB + 1 : 2, o : o + 128]
                            nc.vector.tensor_tensor(
                                out=msl,
                                in0=msl,
                                in1=mask_sb[:, 0:1, 0:128].to_broadcast(
                                    (P, 2, 128)
                                ),
                                op=MULT,
                            )
                    pend.append((eAB2, 2 * sc, otA, otB, nch, pair, jq, W, slots))
                    if len(pend) > 3:
                        emit_pv(pend.pop(0))
                    if filler is not None:
                        # PE filler; emitted last so its DVE ropes queue
                        # behind the masks/norm copies that gate PV
                        if next(filler, StopIteration) is StopIteration:
                            filler = None
                # one extra pull at the window boundary: the next jq's
                # first scores wait a full exp drain, so pre-queue PE work
                if filler is not None and jq < NJQ - 1:
                    if next(filler, StopIteration) is StopIteration:
                        filler = None
            if filler is not None:  # drain leftover projection work
                for _ in filler:
                    pass
                filler = None
            if pair == 2:
                # drain the PV pipeline so pair 2's last AllGather is
                # emitted before the chunk preloads that read it
                while pend:
                    emit_pv(pend.pop(0))
                # xT no longer needed; free it so the wo chunk tiles can
                # take its address range, then preload gathered chunks
                # for pairs 0-2 (their AllGathers are done or in flight).
                proj_ctx.close()
                wos = ctx.enter_context(tc.tile_pool(name="wos", bufs=1))
                wo_sb = wos.tile([P, DCH, OCOLS], bf16)
                for t in range(4):
                    for rr in range(GRP):
                        for h in range(2):
                            chunks[4 * t + rr][h] = wos.tile(
                                [P, 1024], bf16, name=f"ach{t}_{rr}_{h}",
                                tag=f"ach{t}_{rr}_{h}",
                            )
        while pend:
            emit_pv(pend.pop(0))

        # ---- phase C: wo projection (lo/hi halves overlap last AGs) --
        if True:
            for h in range(2):
                for mm in range(8):
                    m = h * 8 + mm
                    mps = pps.tile([P, OCOLS], f32, tag=f"pj{m % 2}", name="mps",
                                   bufs=1)
                    for c2 in range(16):
                        nc.tensor.matmul(
                            mps[:],
                            chunks[c2][h][:, mm * P : (mm + 1) * P],
                            wo_sb[:, c2, :],
                            start=(c2 == 0),
                            stop=(c2 == 15),
                        )
                    osb = wos.tile([P, OCOLS], f32, tag="osb", name="osb", bufs=3)
                    nc.vector.tensor_copy(out=osb[:], in_=mps[:])
                    nc.sync.dma_start(out.ap()[m * P : (m + 1) * P, :], osb[:])


# ---------------------------------------------------------------------------
# host side
# ---------------------------------------------------------------------------

_PERM64 = np.concatenate([np.arange(0, 64, 2), np.arange(1, 64, 2)])


def _qcols(r):
    cols = []
    for t in range(NREP):
        for half in range(2):
            h = (2 * r + half) * NREP + t
            cols.extend(64 * h + _PERM64)
    return np.array(cols)


def _kcols(r):
    cols = []
    for half in range(2):
        g = 2 * r + half
        cols.extend(64 * g + _PERM64)
    return np.array(cols)


def _worows():
    rows = []
    for t in range(NREP):
        for rr in range(GRP):
            for half in range(2):
                h = (2 * rr + half) * NREP + t
                rows.extend(64 * h + np.arange(64))
    return np.array(rows)


def make_in_maps(x, wq, wk, wv, wo, freqs_cos, freqs_sin):
    cosT = np.ascontiguousarray(freqs_cos.T).astype(np.float32)  # (32, S)
    sinT = np.ascontiguousarray(freqs_sin.T).astype(np.float32)
    cos4 = np.ascontiguousarray(np.tile(cosT, (4, 1))).astype(_BF)  # (128, S)
    sin4 = np.ascontiguousarray(
        np.concatenate([-sinT, sinT, -sinT, sinT], axis=0)
    ).astype(_BF)
    cmask = np.triu(np.ones((P, P), dtype=np.float32)).astype(_BF)

    xT = [np.ascontiguousarray(x[b].T).astype(_BF) for b in range(B)]
    wo_perm = wo[_worows(), :]

    in_maps = []
    for c in range(NCORES):
        b, r = c // GRP, c % GRP
        in_maps.append(
            {
                "xT": xT[b],
                "wq": np.ascontiguousarray(wq[:, _qcols(r)]).astype(_BF),
                "wk": np.ascontiguousarray(wk[:, _kcols(r)]).astype(_BF),
                "wv": np.ascontiguousarray(wv[:, 128 * r : 128 * (r + 1)]).astype(_BF),
                "wo": np.ascontiguousarray(
                    wo_perm[:, OCOLS * r : OCOLS * (r + 1)]
                ).astype(_BF),
                "cos4": cos4,
                "sin4": sin4,
                "cmask": cmask,
            }
        )
    return in_maps


_NC_CACHE = None


def _get_nc():
    global _NC_CACHE
    if _NC_CACHE is None:
        _NC_CACHE = build_graph()
    return _NC_CACHE


def kernel(x, wq, wk, wv, wo, freqs_cos, freqs_sin):
    x = np.asarray(x)
    wq = np.asarray(wq)
    wk = np.asarray(wk)
    wv = np.asarray(wv)
    wo = np.asarray(wo)
    freqs_cos = np.asarray(freqs_cos)
    freqs_sin = np.asarray(freqs_sin)

    in_maps = make_in_maps(x, wq, wk, wv, wo, freqs_cos, freqs_sin)
    nc = _get_nc()
    res = run_bass_kernel_spmd(nc, in_maps, core_ids=list(range(NCORES)))

    out = np.empty((B, S, D), dtype=np.float32)
    for c in range(NCORES):
        b, r = c // GRP, c % GRP
        out[b, :, OCOLS * r : OCOLS * (r + 1)] = res.results[c]["out"]
    return out

